# revision 4
# baseline (speedup 1.0000x reference)
"""Windowed attention + dynamic relative position bias on 8 NeuronCores.

Shapes: q,k,v [B=128, H=8, N=256, D=32] f32; pos-MLP width P=16; h=w=16.
Sharding: head-parallel - core c computes head c for all 128 batch windows;
the per-core head is selected purely by the w3 column passed to that core
(program is SPMD-identical).

v2 design (Activation-engine bound; everything else hidden under it):
  - q/k are host-pretransposed: SBUF tiles ARE the qt/kt packs
    [(bi 4, d 32) partition, (g 16, hh 2, n 256) free] - no PE transposes,
    no PSUM evacuation copies.
  - q/k/v loaded in 5 big SWDGE chunks (Pool dispatch ~1us each).
  - QK: row-packed K=32 bf16 matmuls -> S^T [m 128, (j 2, mb 2, n 256)]
    PSUM tiles (2 per half-group); exp on ScalarE -> E bf16 (only Exp/Ln
    are ever used on ScalarE -> single activation-table load).
  - bias: exp(s+b) = exp(s)*exp(b); expb built once via the Toeplitz
    gather (posT -> DRAM -> 16 strided DMAs, half SWDGE half HWDGE ->
    J-matmul un-reversal -> exp); applied as ONE 4D DVE multiply per E
    tile (bf16 all-SBUF, 2x/4x DVE mode).
  - PV with ones-augmented V -> O_ps [128 n, (j 8, e 33)] PSUM, col 32 = Z;
    DMA'd straight from PSUM, normalization (O/Z) done on host.
  - LN rsqrt as exp(-0.5*ln(v+eps)) keeps ScalarE single-table.
"""

import os
import numpy as np

B, H, N, D = 128, 8, 256, 32
P = 16
NCORES = 8
SCALE = float(1.0 / np.sqrt(D))
NGROUPS = 16
NPRE = int(os.environ.get("K_NPRE", "3"))          # prefix groups before expb
DRAIN = int(os.environ.get("K_DRAIN", "2"))        # extra PVs per new half-group
CHUNKS = [(0, 1), (2, 3), (4, 7), (8, 11), (12, 15)]
CHUNK_OF_GROUP = [0, 0, 1, 1, 2, 2, 2, 2, 3, 3, 3, 3, 4, 4, 4, 4]

# big const tile column layout (f32): 12x128 blocks + w3c(8)
_CB = {"w1": 0, "w2": 128, "ident": 256, "bprojt": 384, "g1t": 512,
       "lb1t": 640, "linb1t": 768, "g2t": 896, "lb2t": 1024,
       "linb2t": 1152, "g3t": 1280, "lb3t": 1408, "w3c": 1536}
CONSTW = 1544

_BUILD_CACHE = {}


def _build():
    if "nc" in _BUILD_CACHE:
        return _BUILD_CACHE["nc"]
    import concourse.bacc as bacc
    import concourse.mybir as mybir
    from concourse.tile import TileContext
    from bass_rust import AP

    F32 = mybir.dt.float32
    F32R = mybir.dt.float32r
    BF16 = mybir.dt.bfloat16
    AF = mybir.ActivationFunctionType
    AX = mybir.AxisListType
    ALU = mybir.AluOpType

    nc = bacc.Bacc("TRN2", target_bir_lowering=False, debug=False,
                   num_devices=NCORES)

    # host-prearranged layouts (see build_in_maps):
    # qd [128 p=(bi,d), (g 16, hh 2, n 256)]
    # kd [128 p=(bi,d), (g 16, hh 2, mb 2, m 128)]
    # vd [128 p=m, (b 128, c 2, e 33)] (e==32 -> 1.0)
    qd = nc.dram_tensor("qd", [128, 8192], F32, kind="ExternalInput")
    kd = nc.dram_tensor("kd", [128, 8192], F32, kind="ExternalInput")
    vd = nc.dram_tensor("vd", [128, 8448], F32, kind="ExternalInput")
    biasesT_d = nc.dram_tensor("biasesT", [2, 1024], F32, kind="ExternalInput")
    wproj_d = nc.dram_tensor("wproj", [2, P], F32, kind="ExternalInput")
    b3c_d = nc.dram_tensor("b3c", [8, 1], F32, kind="ExternalInput")
    jmat_d = nc.dram_tensor("jmat", [128, 128], F32, kind="ExternalInput")
    cbig_d = nc.dram_tensor("cbig", [128, CONSTW], F32, kind="ExternalInput")

    posd = nc.dram_tensor("posd", [1, 1024], F32R, kind="Internal")
    out_d = nc.dram_tensor("out", [128, 8448], F32, kind="ExternalOutput")

    with TileContext(nc) as tc:
        with (
            tc.tile_pool(name="const", bufs=1) as constp,
            tc.tile_pool(name="vpool", bufs=1) as vpool,
            tc.tile_pool(name="mlp", bufs=2) as mlpp,
            tc.tile_pool(name="epool", bufs=int(os.environ.get("K_EP", "16"))) as epool,
            tc.tile_pool(name="outp", bufs=int(os.environ.get("K_OUT", "3"))) as outp,
            tc.tile_pool(name="spsum", bufs=int(os.environ.get("K_SB", "2")), space="PSUM") as spsum,
            tc.tile_pool(name="auxpsum", bufs=int(os.environ.get("K_AB", "4")), space="PSUM") as auxpsum,
        ):
            # ---- full-size q/k/v SBUF tiles; chunked loads emitted lazily
            q_all = vpool.tile([128, 8192], BF16)
            k_all = vpool.tile([128, 8192], BF16)
            v_all = vpool.tile([128, 8448], BF16)

            chunk_loaded = [False] * len(CHUNKS)

            def emit_chunk(ci, include_v=True, only_v=False):
                g0, g1 = CHUNKS[ci]
                ng = g1 - g0 + 1
                if not only_v:
                    for t_all, td in ((q_all, qd), (k_all, kd)):
                        nc.gpsimd.dma_start(
                            t_all[:, 512 * g0:512 * (g1 + 1)],
                            AP(td, 512 * g0, [[8192, 128], [1, 512 * ng]]))
                if include_v:
                    nc.gpsimd.dma_start(
                        v_all[:, 528 * g0:528 * (g1 + 1)],
                        AP(vd, 528 * g0, [[8448, 128], [1, 528 * ng]]))

            def ensure_chunk(ci):
                if not chunk_loaded[ci]:
                    chunk_loaded[ci] = True
                    emit_chunk(ci)

            # chunk0 q/k first (feeds first QK), consts next, then the rest
            chunk_loaded[0] = True
            emit_chunk(0, include_v=False)

            # ---------------- constants ----------------
            cbig = constp.tile([128, CONSTW], F32)
            nc.sync.dma_start(cbig[:, :], cbig_d[:, :])
            biasesT = constp.tile([2, 1024], F32)
            nc.sync.dma_start(biasesT[:, :], biasesT_d[:, :])
            wproj = constp.tile([2, P], F32)
            nc.sync.dma_start(wproj[:, :], wproj_d[:, :])
            b3c = constp.tile([8, 1], F32)
            nc.sync.dma_start(b3c[:, :], b3c_d[:, :])
            jmat_f = constp.tile([128, 128], F32)
            nc.sync.dma_start(jmat_f[:, :], jmat_d[:, :])
            jmat_r = constp.tile([128, 128], F32R)
            nc.vector.tensor_copy(jmat_r[:, :], jmat_f[:, :])
            eps_t = constp.tile([128, 1], F32)
            nc.vector.memset(eps_t[:, :], 1e-5)

            emit_chunk(0, only_v=True)
            ensure_chunk(1)

            def cb(nm):
                o = _CB[nm]
                w = 8 if nm == "w3c" else 128
                return cbig[:, o:o + w]

            mlp_env = {}

            def _mlp_layer(x_sb, g_t, beta_t, w_t, linb_t, last=False):
                x3 = x_sb[:, :].rearrange("p (j f) -> p j f", f=16)
                mz = mlpp.tile([128, 8], F32, tag="mz")
                nc.vector.tensor_reduce(mz[:, :], x3, AX.X, ALU.add)
                xc = mlpp.tile([128, 128], F32, tag="xc")
                xc3 = xc[:, :].rearrange("p (j f) -> p j f", f=16)
                # xc' = mz/16 - x  (negated; g tiles are host-negated)
                nc.vector.scalar_tensor_tensor(
                    xc3, mz[:, :].unsqueeze(2).broadcast_to((128, 8, 16)),
                    1.0 / 16.0, x3, ALU.mult, ALU.subtract)
                sq = mlpp.tile([128, 128], F32, tag="sq")
                sq3 = sq[:, :].rearrange("p (j f) -> p j f", f=16)
                nc.vector.tensor_mul(sq3, xc3, xc3)
                vz = mlpp.tile([128, 8], F32, tag="vz")
                nc.vector.tensor_reduce(vz[:, :], sq3, AX.X, ALU.add)
                # rsqrt(v+eps) = exp(-0.5*ln(v+eps)): keeps ScalarE on the
                # ln/exp activation table (no table reloads mid-kernel)
                lnv = mlpp.tile([128, 8], F32, tag="lnv")
                nc.scalar.activation(lnv[:, :], vz[:, :], AF.Ln,
                                     bias=eps_t[:, 0:1], scale=1.0 / 16.0)
                rz = mlpp.tile([128, 8], F32, tag="rz")
                nc.scalar.activation(rz[:, :], lnv[:, :], AF.Exp, scale=-0.5)
                xn = mlpp.tile([128, 128], F32, tag="xn")
                xn3 = xn[:, :].rearrange("p (j f) -> p j f", f=16)
                nc.vector.tensor_mul(
                    xn3, xc3, rz[:, :].unsqueeze(2).broadcast_to((128, 8, 16)))
                y = mlpp.tile([128, 128], F32, tag="y")
                nc.vector.tensor_mul(y[:, :], xn[:, :], g_t[:, :])
                nc.vector.tensor_add(y[:, :], y[:, :], beta_t[:, :])
                yr = mlpp.tile([128, 128], F32, tag="yr")
                nc.vector.tensor_scalar_max(yr[:, :], y[:, :], 0.0)
                pt = auxpsum.tile([128, 512], F32, tag="aux2")
                nc.tensor.matmul(pt[:, :128], yr[:, :], cb("ident"),
                                 is_transpose=True)
                yT = mlpp.tile([128, 128], F32, tag="yT")
                nc.vector.tensor_copy(yT[:, :], pt[:, :128])
                if last:
                    return yT
                px = auxpsum.tile([128, 512], F32, tag="aux2")
                nc.tensor.matmul(px[:, :128], yT[:, :], w_t)
                xnext = mlpp.tile([128, 128], F32, tag="xnext")
                nc.vector.tensor_add(xnext[:, :], px[:, :128], linb_t)
                return xnext

            def emit_mlp_stage(stage):
                """0=x0, 1..3=LN layers, 4=pos->DRAM->gather (sets btrev)."""
                env = mlp_env
                if stage == 0:
                    px0 = auxpsum.tile([128, 512], F32, tag="aux2")
                    for j in range(8):
                        nc.tensor.matmul(px0[:, 16 * j:16 * j + 16],
                                         biasesT[:, 128 * j:128 * j + 128],
                                         wproj[:, :])
                    x0 = mlpp.tile([128, 128], F32, tag="x0")
                    nc.vector.tensor_add(x0[:, :], px0[:, :128], cb("bprojt"))
                    env["x0"] = x0
                    return
                if stage == 1:
                    env["x1"] = _mlp_layer(env["x0"], cb("g1t"), cb("lb1t"),
                                           cb("w1"), cb("linb1t"))
                    return
                if stage == 2:
                    env["x2"] = _mlp_layer(env["x1"], cb("g2t"), cb("lb2t"),
                                           cb("w2"), cb("linb2t"))
                    return
                if stage == 3:
                    env["y3T"] = _mlp_layer(env["x2"], cb("g3t"), cb("lb3t"),
                                            None, None, last=True)
                    return
                # stage 4: posT -> DRAM -> Toeplitz gather (reversed m)
                pos_ps = auxpsum.tile([128, 512], F32, tag="aux2")
                nc.tensor.matmul(pos_ps[0:8, :128], cb("w3c"), env["y3T"][:, :])
                pos_sb = constp.tile([8, 128], F32R)
                nc.vector.tensor_scalar_add(pos_sb[:, :], pos_ps[0:8, :128],
                                            b3c[:, 0:1])
                nc.sync.dma_start(AP(posd, 0, [[128, 8], [1, 128]]),
                                  pos_sb[:, :])
                btrev = []
                for mbp in range(2):
                    bt = constp.tile([128, 256], F32R, tag=f"btrev{mbp}")
                    btrev.append(bt)
                    for a in range(8):
                        src = AP(posd, 31 * (8 * mbp + a),
                                 [[1, 16], [31, 16], [1, 16]])
                        dst = bt[16 * a:16 * a + 16, :].rearrange(
                            "b (c e) -> b c e", e=16)
                        # split dispatch across HWDGE and Pool/SWDGE to
                        # halve the serialized gather latency
                        if a % 2 == 0:
                            nc.sync.dma_start(dst, src)
                        else:
                            nc.gpsimd.dma_start(dst, src)
                env["btrev"] = btrev

            def emit_expb():
                btrev = mlp_env["btrev"]
                expb = constp.tile([128, 512], BF16)
                for mb in range(2):
                    pe_ = auxpsum.tile([128, 512], F32, tag="aux2",
                                       name=f"pexpb{mb}")
                    nc.tensor.matmul(pe_[:, :256], jmat_r[:, :],
                                     btrev[1 - mb][:, :])
                    nc.scalar.activation(expb[:, 256 * mb:256 * mb + 256],
                                         pe_[:, :256], AF.Exp, scale=SCALE)
                return expb

            # --- main pipeline over 32 half-groups ---
            def emit_qk_exp(g, hh):
                """QK matmuls + exp for half-group (g, hh) -> epair."""
                ho = 512 * g + 256 * hh
                epair = []
                for half in range(2):
                    sp = spsum.tile([128, 1024], F32, tag="S",
                                    name=f"s{g}_{hh}_{half}")
                    for bi2 in range(2):
                        bi = 2 * half + bi2
                        fo = 512 * bi2
                        for mb in range(2):
                            nc.tensor.matmul(
                                sp[:, fo + 256 * mb:fo + 256 * mb + 256],
                                k_all[32 * bi:32 * bi + 32,
                                      ho + 128 * mb:ho + 128 * mb + 128],
                                q_all[32 * bi:32 * bi + 32, ho:ho + 256],
                                tile_position=(32 * bi, 0),
                                start=True, stop=True)
                    e = epool.tile([128, 1024], BF16, tag="E",
                                   name=f"e{g}_{hh}_{half}")
                    nc.scalar.activation(e[:, :], sp[:, :], AF.Exp,
                                         scale=SCALE)
                    epair.append(e)
                return epair

            def emit_emul(expb, epair):
                for e in epair:
                    e4 = e[:, :].rearrange("p (j mb n) -> p j mb n",
                                           mb=2, n=256)
                    nc.vector.tensor_mul(
                        e4, e4,
                        expb[:, :].rearrange("p (mb n) -> p mb n", n=256)
                        .unsqueeze(1).broadcast_to((128, 2, 2, 256)))

            def emit_pv_out(g, hh, epair):
                o_ps = auxpsum.tile([128, 264], F32, tag="aux2",
                                    name=f"ops{g}_{hh}")
                for bi in range(4):
                    e = epair[bi // 2]
                    fo = 512 * (bi % 2)
                    vb = 66 * (8 * g + 4 * hh + bi)
                    for nb in range(2):
                        j = 2 * bi + nb
                        for c in range(2):
                            nc.tensor.matmul(
                                o_ps[:, 33 * j:33 * j + 33],
                                e[:, fo + 256 * c + 128 * nb:
                                  fo + 256 * c + 128 * nb + 128],
                                v_all[:, vb + 33 * c:vb + 33 * c + 33],
                                start=(c == 0), stop=(c == 1))
                osb = outp.tile([128, 264], F32, tag="osb",
                                name=f"osb{g}_{hh}")
                nc.vector.tensor_copy(osb[:, :], o_ps[:, :])
                nc.sync.dma_start(
                    AP(out_d, 264 * (2 * g + hh), [[8448, 128], [1, 264]]),
                    osb[:, :264])

            # ---- schedule ----
            pend = []           # (g, hh, epair) awaiting bias-mul + PV

            emit_mlp_stage(0)
            for g in range(NPRE):
                ensure_chunk(CHUNK_OF_GROUP[min(g + 2, NGROUPS - 1)])
                for hh in range(2):
                    pend.append((g, hh, emit_qk_exp(g, hh)))
                if g + 1 < 4:
                    emit_mlp_stage(g + 1)
            for st in range(NPRE + 1, 5):
                emit_mlp_stage(st)
            expb = emit_expb()

            for g in range(NPRE, NGROUPS):
                ensure_chunk(CHUNK_OF_GROUP[min(g + 2, NGROUPS - 1)])
                for hh in range(2):
                    pend.append((g, hh, emit_qk_exp(g, hh)))
                    for _ in range(DRAIN):
                        if len(pend) > 1:
                            pg, phh, pep = pend.pop(0)
                            emit_emul(expb, pep)
                            emit_pv_out(pg, phh, pep)
            for pg, phh, pep in pend:
                emit_emul(expb, pep)
                emit_pv_out(pg, phh, pep)

    nc.compile()
    _BUILD_CACHE["nc"] = nc
    return nc


def _host_constants():
    hh, ww = 16, 16
    bh, bw = np.meshgrid(np.arange(1 - hh, hh), np.arange(1 - ww, ww),
                         indexing="ij")
    biases = np.stack([bh, bw], -1).reshape(-1, 2).astype(np.float32)
    biasesT = np.zeros((2, 1024), np.float32)
    biasesT[:, :961] = biases.T
    return biasesT


def _blk8(w16):
    cout = w16.shape[1]
    blk = np.zeros((128, 8 * cout), np.float32)
    for j in range(8):
        blk[16 * j:16 * j + 16, cout * j:cout * j + cout] = w16
    return np.ascontiguousarray(blk)


def _tile16(vec):
    return np.ascontiguousarray(
        np.tile(np.asarray(vec, np.float32), (128, 8)))


def build_in_maps(inputs):
    q = np.asarray(inputs["q"], np.float32)
    k = np.asarray(inputs["k"], np.float32)
    v = np.asarray(inputs["v"], np.float32)
    hh = int(np.asarray(inputs["h"]))
    ww = int(np.asarray(inputs["w"]))
    assert hh == 16 and ww == 16, (hh, ww)
    f32 = lambda name: np.asarray(inputs[name], np.float32)
    w3 = f32("w3")
    b3 = f32("b3")
    sqrtD = np.float32(np.sqrt(D))

    cblk = {
        "w1": _blk8(f32("w1")), "w2": _blk8(f32("w2")),
        "ident": np.eye(128, dtype=np.float32),
        "bprojt": _tile16(f32("b_proj")),
        "g1t": -_tile16(f32("ln1_g")), "lb1t": _tile16(f32("ln1_b")),
        "linb1t": _tile16(f32("b1")),
        "g2t": -_tile16(f32("ln2_g")), "lb2t": _tile16(f32("ln2_b")),
        "linb2t": _tile16(f32("b2")),
        "g3t": -_tile16(f32("ln3_g")), "lb3t": _tile16(f32("ln3_b")),
    }
    shared = {
        "biasesT": _host_constants(),
        "wproj": f32("w_proj"),
        "jmat": np.eye(128, dtype=np.float32)[::-1].copy(),
    }

    def q_layout(x):
        # [128 w, 256 n, 32 d] -> [128 p=(bi,d), (g, hh, n)]
        x5 = x.reshape(16, 2, 4, 256, 32)           # g hh bi n d
        return np.ascontiguousarray(
            x5.transpose(2, 4, 0, 1, 3).reshape(128, 8192))

    def k_layout(x):
        # [128 w, 256 m, 32 d] -> [128 p=(bi,d), (g, hh, mb, m)]
        x6 = x.reshape(16, 2, 4, 2, 128, 32)        # g hh bi mb m d
        return np.ascontiguousarray(
            x6.transpose(2, 5, 0, 1, 3, 4).reshape(128, 8192))

    def v_layout(x):
        # [128 p=m, (b 128, c 2, e 33)]; e==32 -> 1.0
        v4 = x.reshape(128, 2, 128, 32)             # b c p e
        out = np.ones((128, 128, 2, 33), np.float32)
        out[:, :, :, :32] = v4.transpose(2, 0, 1, 3)
        return np.ascontiguousarray(out.reshape(128, 8448))

    in_maps = []
    for c in range(NCORES):
        cbig = np.empty((128, CONSTW), np.float32)
        for nm, off in _CB.items():
            if nm == "w3c":
                cbig[:, off:off + 8] = _blk8(w3[:, c:c + 1] * sqrtD)
            else:
                cbig[:, off:off + 128] = cblk[nm]
        m = dict(shared)
        m["cbig"] = np.ascontiguousarray(cbig)
        m["qd"] = q_layout(q[:, c])
        m["kd"] = k_layout(k[:, c])
        m["vd"] = v_layout(v[:, c])
        m["b3c"] = np.full((8, 1), b3[c], np.float32) * sqrtD
        in_maps.append(m)
    return in_maps


def unshard_out(raw):
    # raw [128 p, (g 16, hh 2, bi 4, nb 2, e 33)] -> normalized [B, N, D]
    r6 = raw.reshape(128, 16, 2, 4, 2, 33)          # p g hh bi nb e
    o = r6.transpose(1, 2, 3, 4, 0, 5).reshape(128, 256, 33)
    return o[:, :, :32] / o[:, :, 32:33]


def kernel(**inputs):
    from concourse.bass_utils import run_bass_kernel_spmd

    nc = _build()
    in_maps = build_in_maps(inputs)
    res = run_bass_kernel_spmd(nc, in_maps, core_ids=list(range(NCORES)))
    out = np.empty((B, H, N, D), np.float32)
    for c in range(NCORES):
        out[:, c] = unshard_out(res.results[c]["out"])
    return out


# revision 11
# speedup vs baseline: 1.0682x; 1.0682x over previous
"""Windowed attention + dynamic relative position bias on 8 NeuronCores.

Shapes: q,k,v [B=128, H=8, N=256, D=32] f32; pos-MLP width P=16; h=w=16.
Sharding: head-parallel - core c computes head c for all 128 batch windows;
the per-core head is selected purely by the w3 column passed to that core
(program is SPMD-identical).

v2 design (Activation-engine bound; everything else hidden under it):
  - q/k are host-pretransposed: SBUF tiles ARE the qt/kt packs
    [(bi 4, d 32) partition, (g 16, hh 2, n 256) free] - no PE transposes,
    no PSUM evacuation copies.
  - q/k/v loaded in 5 big SWDGE chunks (Pool dispatch ~1us each).
  - QK: row-packed K=32 bf16 matmuls -> S^T [m 128, (j 2, mb 2, n 256)]
    PSUM tiles (2 per half-group); exp on ScalarE -> E bf16 (only Exp/Ln
    are ever used on ScalarE -> single activation-table load).
  - bias: exp(s+b) = exp(s)*exp(b); expb built once via the Toeplitz
    gather (posT -> DRAM -> 16 strided DMAs, half SWDGE half HWDGE ->
    J-matmul un-reversal -> exp); applied as ONE 4D DVE multiply per E
    tile (bf16 all-SBUF, 2x/4x DVE mode).
  - PV with ones-augmented V -> O_ps [128 n, (j 8, e 33)] PSUM, col 32 = Z;
    normalize fused into the PSUM evacuation (reciprocal + broadcast mul).
  - LN rsqrt on DVE (bitcast magic + 2 Newton steps) -> ScalarE only ever
    runs Exp -> exactly one activation-table load, no thrash.
  - pipeline: PV emitted 2 half-groups behind QK/exp so its deps are
    resolved before the PE sequencer reaches it (4-deep wait queue).
"""

import os
import numpy as np

B, H, N, D = 128, 8, 256, 32
P = 16
NCORES = 8
SCALE = float(1.0 / np.sqrt(D))
NGROUPS = 16
NPRE = int(os.environ.get("K_NPRE", "3"))          # prefix groups before expb
DRAIN = int(os.environ.get("K_DRAIN", "2"))        # extra PVs per new half-group
CHUNKS = [(0, 1), (2, 3), (4, 7), (8, 11), (12, 15)]
CHUNK_OF_GROUP = [0, 0, 1, 1, 2, 2, 2, 2, 3, 3, 3, 3, 4, 4, 4, 4]

# big const tile column layout (f32): 12x128 blocks + w3c(8)
_CB = {"w1": 0, "w2": 128, "ident": 256, "bprojt": 384, "g1t": 512,
       "lb1t": 640, "linb1t": 768, "g2t": 896, "lb2t": 1024,
       "linb2t": 1152, "g3t": 1280, "lb3t": 1408, "w3c": 1536}
CONSTW = 1544

_BUILD_CACHE = {}


def _build():
    if "nc" in _BUILD_CACHE:
        return _BUILD_CACHE["nc"]
    import concourse.bacc as bacc
    import concourse.mybir as mybir
    from concourse.tile import TileContext
    from bass_rust import AP

    F32 = mybir.dt.float32
    F32R = mybir.dt.float32r
    BF16 = mybir.dt.bfloat16
    AF = mybir.ActivationFunctionType
    AX = mybir.AxisListType
    ALU = mybir.AluOpType

    nc = bacc.Bacc("TRN2", target_bir_lowering=False, debug=False,
                   num_devices=NCORES)

    # host-prearranged layouts (see build_in_maps):
    # qd [128 p=(bi,d), (g 16, hh 2, n 256)]
    # kd [128 p=(bi,d), (g 16, hh 2, mb 2, m 128)]
    # vd [128 p=m, (b 128, c 2, e 33)] (e==32 -> 1.0)
    qd = nc.dram_tensor("qd", [128, 8192], F32, kind="ExternalInput")
    kd = nc.dram_tensor("kd", [128, 8192], F32, kind="ExternalInput")
    vd = nc.dram_tensor("vd", [128, 8448], F32, kind="ExternalInput")
    biasesT_d = nc.dram_tensor("biasesT", [2, 1024], F32, kind="ExternalInput")
    wproj_d = nc.dram_tensor("wproj", [2, P], F32, kind="ExternalInput")
    b3c_d = nc.dram_tensor("b3c", [8, 1], F32, kind="ExternalInput")
    jmat_d = nc.dram_tensor("jmat", [128, 128], F32, kind="ExternalInput")
    cbig_d = nc.dram_tensor("cbig", [128, CONSTW], F32, kind="ExternalInput")

    posd = nc.dram_tensor("posd", [1, 1024], F32R, kind="Internal")
    out_d = nc.dram_tensor("out", [128, 8192], F32, kind="ExternalOutput")
    I32 = mybir.dt.int32

    with TileContext(nc) as tc:
        with (
            tc.tile_pool(name="const", bufs=1) as constp,
            tc.tile_pool(name="vpool", bufs=1) as vpool,
            tc.tile_pool(name="mlp", bufs=2) as mlpp,
            tc.tile_pool(name="epool", bufs=int(os.environ.get("K_EP", "16"))) as epool,
            tc.tile_pool(name="outp", bufs=int(os.environ.get("K_OUT", "3"))) as outp,
            tc.tile_pool(name="spsum", bufs=int(os.environ.get("K_SB", "2")), space="PSUM") as spsum,
            tc.tile_pool(name="auxpsum", bufs=int(os.environ.get("K_AB", "4")), space="PSUM") as auxpsum,
        ):
            # ---- full-size q/k/v SBUF tiles; chunked loads emitted lazily
            q_all = vpool.tile([128, 8192], BF16)
            k_all = vpool.tile([128, 8192], BF16)
            v_all = vpool.tile([128, 8448], BF16)

            chunk_loaded = [False] * len(CHUNKS)

            def emit_chunk(ci, include_v=True, only_v=False):
                g0, g1 = CHUNKS[ci]
                ng = g1 - g0 + 1
                if not only_v:
                    for t_all, td in ((q_all, qd), (k_all, kd)):
                        nc.gpsimd.dma_start(
                            t_all[:, 512 * g0:512 * (g1 + 1)],
                            AP(td, 512 * g0, [[8192, 128], [1, 512 * ng]]))
                if include_v:
                    nc.gpsimd.dma_start(
                        v_all[:, 528 * g0:528 * (g1 + 1)],
                        AP(vd, 528 * g0, [[8448, 128], [1, 528 * ng]]))

            def ensure_chunk(ci):
                if not chunk_loaded[ci]:
                    chunk_loaded[ci] = True
                    emit_chunk(ci)

            # chunk0 q/k first (feeds first QK), consts next, then the rest
            chunk_loaded[0] = True
            emit_chunk(0, include_v=False)

            # ---------------- constants ----------------
            cbig = constp.tile([128, CONSTW], F32)
            nc.sync.dma_start(cbig[:, :], cbig_d[:, :])
            biasesT = constp.tile([2, 1024], F32)
            nc.sync.dma_start(biasesT[:, :], biasesT_d[:, :])
            wproj = constp.tile([2, P], F32)
            nc.sync.dma_start(wproj[:, :], wproj_d[:, :])
            b3c = constp.tile([8, 1], F32)
            nc.sync.dma_start(b3c[:, :], b3c_d[:, :])
            jmat_f = constp.tile([128, 128], F32)
            nc.sync.dma_start(jmat_f[:, :], jmat_d[:, :])
            jmat_r = constp.tile([128, 128], F32R)
            nc.vector.tensor_copy(jmat_r[:, :], jmat_f[:, :])
            magic_t = constp.tile([128, 8], I32)
            nc.vector.memset(magic_t[:, :], 0x5F3759DF)

            emit_chunk(0, only_v=True)
            ensure_chunk(1)

            def cb(nm):
                o = _CB[nm]
                w = 8 if nm == "w3c" else 128
                return cbig[:, o:o + w]

            mlp_env = {}

            def _mlp_layer(x_sb, g_t, beta_t, w_t, linb_t, last=False):
                x3 = x_sb[:, :].rearrange("p (j f) -> p j f", f=16)
                mz = mlpp.tile([128, 8], F32, tag="mz")
                nc.vector.tensor_reduce(mz[:, :], x3, AX.X, ALU.add)
                xc = mlpp.tile([128, 128], F32, tag="xc")
                xc3 = xc[:, :].rearrange("p (j f) -> p j f", f=16)
                # xc' = mz/16 - x  (negated; g tiles are host-negated)
                nc.vector.scalar_tensor_tensor(
                    xc3, mz[:, :].unsqueeze(2).broadcast_to((128, 8, 16)),
                    1.0 / 16.0, x3, ALU.mult, ALU.subtract)
                sq = mlpp.tile([128, 128], F32, tag="sq")
                sq3 = sq[:, :].rearrange("p (j f) -> p j f", f=16)
                nc.vector.tensor_mul(sq3, xc3, xc3)
                vz = mlpp.tile([128, 8], F32, tag="vz")
                nc.vector.tensor_reduce(vz[:, :], sq3, AX.X, ALU.add)
                # rsqrt(v/16 + eps) fully on DVE (bit-magic + 2 Newton
                # steps) so ScalarE only ever runs Exp (one act table).
                w = mlpp.tile([128, 8], F32, tag="w")
                nc.vector.tensor_scalar(w[:, :], vz[:, :], 1.0 / 16.0, 1e-5,
                                        ALU.mult, ALU.add)
                sh = mlpp.tile([128, 8], I32, tag="sh")
                nc.vector.tensor_single_scalar(sh[:, :],
                                               w[:, :].bitcast(I32), 1,
                                               ALU.arith_shift_right)
                yi = mlpp.tile([128, 8], I32, tag="yi")
                nc.vector.tensor_sub(yi[:, :], magic_t[:, :], sh[:, :])
                y0 = yi[:, :].bitcast(F32)
                rz = None
                for it in range(2):
                    t = mlpp.tile([128, 8], F32, tag=f"nt{it}")
                    nc.vector.tensor_mul(t[:, :], w[:, :], y0)
                    nc.vector.tensor_mul(t[:, :], t[:, :], y0)
                    nc.vector.tensor_scalar(t[:, :], t[:, :], -0.5, 1.5,
                                            ALU.mult, ALU.add)
                    y1 = mlpp.tile([128, 8], F32, tag=f"ny{it}")
                    nc.vector.tensor_mul(y1[:, :], y0, t[:, :])
                    y0 = y1[:, :]
                    rz = y1
                xn = mlpp.tile([128, 128], F32, tag="xn")
                xn3 = xn[:, :].rearrange("p (j f) -> p j f", f=16)
                nc.vector.tensor_mul(
                    xn3, xc3, rz[:, :].unsqueeze(2).broadcast_to((128, 8, 16)))
                y = mlpp.tile([128, 128], F32, tag="y")
                nc.vector.tensor_mul(y[:, :], xn[:, :], g_t[:, :])
                nc.vector.tensor_add(y[:, :], y[:, :], beta_t[:, :])
                yr = mlpp.tile([128, 128], F32, tag="yr")
                nc.vector.tensor_scalar_max(yr[:, :], y[:, :], 0.0)
                pt = auxpsum.tile([128, 512], F32, tag="aux2")
                nc.tensor.matmul(pt[:, :128], yr[:, :], cb("ident"),
                                 is_transpose=True)
                yT = mlpp.tile([128, 128], F32, tag="yT")
                nc.vector.tensor_copy(yT[:, :], pt[:, :128])
                if last:
                    return yT
                px = auxpsum.tile([128, 512], F32, tag="aux2")
                nc.tensor.matmul(px[:, :128], yT[:, :], w_t)
                xnext = mlpp.tile([128, 128], F32, tag="xnext")
                nc.vector.tensor_add(xnext[:, :], px[:, :128], linb_t)
                return xnext

            def emit_mlp_stage(stage):
                """0=x0, 1..3=LN layers, 4=pos->DRAM->gather (sets btrev)."""
                env = mlp_env
                if stage == 0:
                    px0 = auxpsum.tile([128, 512], F32, tag="aux2")
                    for j in range(8):
                        nc.tensor.matmul(px0[:, 16 * j:16 * j + 16],
                                         biasesT[:, 128 * j:128 * j + 128],
                                         wproj[:, :])
                    x0 = mlpp.tile([128, 128], F32, tag="x0")
                    nc.vector.tensor_add(x0[:, :], px0[:, :128], cb("bprojt"))
                    env["x0"] = x0
                    return
                if stage == 1:
                    env["x1"] = _mlp_layer(env["x0"], cb("g1t"), cb("lb1t"),
                                           cb("w1"), cb("linb1t"))
                    return
                if stage == 2:
                    env["x2"] = _mlp_layer(env["x1"], cb("g2t"), cb("lb2t"),
                                           cb("w2"), cb("linb2t"))
                    return
                if stage == 3:
                    env["y3T"] = _mlp_layer(env["x2"], cb("g3t"), cb("lb3t"),
                                            None, None, last=True)
                    return
                # stage 4: posT -> DRAM -> Toeplitz gather (reversed m)
                pos_ps = auxpsum.tile([128, 512], F32, tag="aux2")
                nc.tensor.matmul(pos_ps[0:8, :128], cb("w3c"), env["y3T"][:, :])
                pos_sb = constp.tile([8, 128], F32R)
                nc.vector.tensor_scalar_add(pos_sb[:, :], pos_ps[0:8, :128],
                                            b3c[:, 0:1])
                nc.sync.dma_start(AP(posd, 0, [[128, 8], [1, 128]]),
                                  pos_sb[:, :])
                btrev = []
                for mbp in range(2):
                    bt = constp.tile([128, 256], F32R, tag=f"btrev{mbp}")
                    btrev.append(bt)
                    for a in range(8):
                        src = AP(posd, 31 * (8 * mbp + a),
                                 [[1, 16], [31, 16], [1, 16]])
                        dst = bt[16 * a:16 * a + 16, :].rearrange(
                            "b (c e) -> b c e", e=16)
                        # split dispatch across HWDGE and Pool/SWDGE to
                        # halve the serialized gather latency
                        if a % 2 == 0:
                            nc.sync.dma_start(dst, src)
                        else:
                            nc.gpsimd.dma_start(dst, src)
                env["btrev"] = btrev

            def emit_expb():
                btrev = mlp_env["btrev"]
                expb = constp.tile([128, 512], BF16)
                for mb in range(2):
                    pe_ = auxpsum.tile([128, 512], F32, tag="aux2",
                                       name=f"pexpb{mb}")
                    nc.tensor.matmul(pe_[:, :256], jmat_r[:, :],
                                     btrev[1 - mb][:, :])
                    nc.scalar.activation(expb[:, 256 * mb:256 * mb + 256],
                                         pe_[:, :256], AF.Exp, scale=SCALE)
                return expb

            # --- main pipeline over 32 half-groups ---
            def emit_qk_exp(g, hh):
                """QK matmuls + exp for half-group (g, hh) -> epair."""
                ho = 512 * g + 256 * hh
                epair = []
                for half in range(2):
                    sp = spsum.tile([128, 1024], F32, tag="S",
                                    name=f"s{g}_{hh}_{half}")
                    for bi2 in range(2):
                        bi = 2 * half + bi2
                        fo = 512 * bi2
                        for mb in range(2):
                            nc.tensor.matmul(
                                sp[:, fo + 256 * mb:fo + 256 * mb + 256],
                                k_all[32 * bi:32 * bi + 32,
                                      ho + 128 * mb:ho + 128 * mb + 128],
                                q_all[32 * bi:32 * bi + 32, ho:ho + 256],
                                tile_position=(32 * bi, 0),
                                start=True, stop=True)
                    e = epool.tile([128, 1024], BF16, tag="E",
                                   name=f"e{g}_{hh}_{half}")
                    nc.scalar.activation(e[:, :], sp[:, :], AF.Exp,
                                         scale=SCALE)
                    epair.append(e)
                return epair

            def emit_emul(expb, epair):
                for e in epair:
                    e4 = e[:, :].rearrange("p (j mb n) -> p j mb n",
                                           mb=2, n=256)
                    nc.vector.tensor_mul(
                        e4, e4,
                        expb[:, :].rearrange("p (mb n) -> p mb n", n=256)
                        .unsqueeze(1).broadcast_to((128, 2, 2, 256)))

            def emit_pv_out(g, hh, epair):
                o_ps = auxpsum.tile([128, 264], F32, tag="aux2",
                                    name=f"ops{g}_{hh}")
                for bi in range(4):
                    e = epair[bi // 2]
                    fo = 512 * (bi % 2)
                    vb = 66 * (8 * g + 4 * hh + bi)
                    for nb in range(2):
                        j = 2 * bi + nb
                        for c in range(2):
                            nc.tensor.matmul(
                                o_ps[:, 33 * j:33 * j + 33],
                                e[:, fo + 256 * c + 128 * nb:
                                  fo + 256 * c + 128 * nb + 128],
                                v_all[:, vb + 33 * c:vb + 33 * c + 33],
                                start=(c == 0), stop=(c == 1))
                rz = outp.tile([128, 8], F32, tag="rz", name=f"rz{g}_{hh}")
                o3 = o_ps[:, :].rearrange("p (j e) -> p j e", e=33)
                nc.vector.reciprocal(rz[:, :], o3[:, :, 32:33])
                osb = outp.tile([128, 256], F32, tag="osb",
                                name=f"osb{g}_{hh}")
                nc.vector.tensor_mul(
                    osb[:, :].rearrange("p (j e) -> p j e", e=32),
                    o3[:, :, :32],
                    rz[:, :].unsqueeze(2).broadcast_to((128, 8, 32)))
                nc.sync.dma_start(
                    AP(out_d, 256 * (2 * g + hh), [[8192, 128], [1, 256]]),
                    osb[:, :])

            # ---- schedule ----
            # unmul: exp'd, bias-mul not yet emitted; unpv: mul'd, PV not
            # yet emitted. Steady state: mul lags 1 half-group, PV lags 2,
            # and PVs are emitted BEFORE the new QK so their deps are
            # already satisfied when the PE sequencer reaches them.
            unmul = []
            unpv = []

            emit_mlp_stage(0)
            for g in range(NPRE):
                ensure_chunk(CHUNK_OF_GROUP[min(g + 2, NGROUPS - 1)])
                for hh in range(2):
                    unmul.append((g, hh, emit_qk_exp(g, hh)))
                if g + 1 < 4:
                    emit_mlp_stage(g + 1)
            for st in range(NPRE + 1, 5):
                emit_mlp_stage(st)
            expb = emit_expb()

            for g in range(NPRE, NGROUPS):
                ensure_chunk(CHUNK_OF_GROUP[min(g + 2, NGROUPS - 1)])
                for hh in range(2):
                    for _ in range(DRAIN):
                        if unpv:
                            emit_pv_out(*unpv.pop(0))
                    for _ in range(DRAIN):
                        if unmul:
                            item = unmul.pop(0)
                            emit_emul(expb, item[2])
                            unpv.append(item)
                    unmul.append((g, hh, emit_qk_exp(g, hh)))
            for item in unmul:
                emit_emul(expb, item[2])
                unpv.append(item)
            for item in unpv:
                emit_pv_out(*item)

    nc.compile()
    _BUILD_CACHE["nc"] = nc
    return nc


def _host_constants():
    hh, ww = 16, 16
    bh, bw = np.meshgrid(np.arange(1 - hh, hh), np.arange(1 - ww, ww),
                         indexing="ij")
    biases = np.stack([bh, bw], -1).reshape(-1, 2).astype(np.float32)
    biasesT = np.zeros((2, 1024), np.float32)
    biasesT[:, :961] = biases.T
    return biasesT


def _blk8(w16):
    cout = w16.shape[1]
    blk = np.zeros((128, 8 * cout), np.float32)
    for j in range(8):
        blk[16 * j:16 * j + 16, cout * j:cout * j + cout] = w16
    return np.ascontiguousarray(blk)


def _tile16(vec):
    return np.ascontiguousarray(
        np.tile(np.asarray(vec, np.float32), (128, 8)))


def build_in_maps(inputs):
    q = np.asarray(inputs["q"], np.float32)
    k = np.asarray(inputs["k"], np.float32)
    v = np.asarray(inputs["v"], np.float32)
    hh = int(np.asarray(inputs["h"]))
    ww = int(np.asarray(inputs["w"]))
    assert hh == 16 and ww == 16, (hh, ww)
    f32 = lambda name: np.asarray(inputs[name], np.float32)
    w3 = f32("w3")
    b3 = f32("b3")
    sqrtD = np.float32(np.sqrt(D))

    cblk = {
        "w1": _blk8(f32("w1")), "w2": _blk8(f32("w2")),
        "ident": np.eye(128, dtype=np.float32),
        "bprojt": _tile16(f32("b_proj")),
        "g1t": -_tile16(f32("ln1_g")), "lb1t": _tile16(f32("ln1_b")),
        "linb1t": _tile16(f32("b1")),
        "g2t": -_tile16(f32("ln2_g")), "lb2t": _tile16(f32("ln2_b")),
        "linb2t": _tile16(f32("b2")),
        "g3t": -_tile16(f32("ln3_g")), "lb3t": _tile16(f32("ln3_b")),
    }
    shared = {
        "biasesT": _host_constants(),
        "wproj": f32("w_proj"),
        "jmat": np.eye(128, dtype=np.float32)[::-1].copy(),
    }

    def q_layout(x):
        # [128 w, 256 n, 32 d] -> [128 p=(bi,d), (g, hh, n)]
        x5 = x.reshape(16, 2, 4, 256, 32)           # g hh bi n d
        return np.ascontiguousarray(
            x5.transpose(2, 4, 0, 1, 3).reshape(128, 8192))

    def k_layout(x):
        # [128 w, 256 m, 32 d] -> [128 p=(bi,d), (g, hh, mb, m)]
        x6 = x.reshape(16, 2, 4, 2, 128, 32)        # g hh bi mb m d
        return np.ascontiguousarray(
            x6.transpose(2, 5, 0, 1, 3, 4).reshape(128, 8192))

    def v_layout(x):
        # [128 p=m, (b 128, c 2, e 33)]; e==32 -> 1.0
        v4 = x.reshape(128, 2, 128, 32)             # b c p e
        out = np.ones((128, 128, 2, 33), np.float32)
        out[:, :, :, :32] = v4.transpose(2, 0, 1, 3)
        return np.ascontiguousarray(out.reshape(128, 8448))

    in_maps = []
    for c in range(NCORES):
        cbig = np.empty((128, CONSTW), np.float32)
        for nm, off in _CB.items():
            if nm == "w3c":
                cbig[:, off:off + 8] = _blk8(w3[:, c:c + 1] * sqrtD)
            else:
                cbig[:, off:off + 128] = cblk[nm]
        m = dict(shared)
        m["cbig"] = np.ascontiguousarray(cbig)
        m["qd"] = q_layout(q[:, c])
        m["kd"] = k_layout(k[:, c])
        m["vd"] = v_layout(v[:, c])
        m["b3c"] = np.full((8, 1), b3[c], np.float32) * sqrtD
        in_maps.append(m)
    return in_maps


def unshard_out(raw):
    # raw [128 p, (g 16, hh 2, bi 4, nb 2, e 32)] -> [B, N, D]
    r6 = raw.reshape(128, 16, 2, 4, 2, 32)          # p g hh bi nb e
    return np.ascontiguousarray(
        r6.transpose(1, 2, 3, 4, 0, 5).reshape(128, 256, 32))


def kernel(**inputs):
    from concourse.bass_utils import run_bass_kernel_spmd

    nc = _build()
    in_maps = build_in_maps(inputs)
    res = run_bass_kernel_spmd(nc, in_maps, core_ids=list(range(NCORES)))
    out = np.empty((B, H, N, D), np.float32)
    for c in range(NCORES):
        out[:, c] = unshard_out(res.results[c]["out"])
    return out
